# revision 2
# baseline (speedup 1.0000x reference)
"""Nearest-neighbor attention Trainium2 kernel (8 NeuronCores).

Strategy:
  - Host: compute kNN neighbor lists from the voxel mask (tiny), pick a
    spatial ordering (recursive bisection) of the 4096 queries, and build
    per-128-query-tile lists of 128-token kv blocks containing neighbors,
    plus {0,1} masks selecting true neighbor entries inside those blocks.
  - Device (SPMD over 8 cores, tensor-parallel over heads: 2 heads/core):
    QKV projections as float32r matmuls (x pre-transposed on host), then
    block-sparse attention: scores^T = kT.T-chunk @ qT-tile per kv block,
    exp via ScalarE, mask multiply, PV via matmul with an appended
    ones-column for the softmax denominator, then per-row normalization.
  - Host: gather per-core outputs, undo the query permutation.
"""

import sys
import os
import numpy as np

sys.path.insert(0, "/opt/trn_rl_repo")

B, N, D = 2, 4096, 1024
H, KNN = 16, 16
HD = D // H          # 64
NCORES = 8
HPC = H // NCORES    # heads per core = 2
NT = N // 128        # 32 query tiles per batch
DC = D // 128        # 8 contraction chunks
TB = 512             # projection token-block size
NBLK = (B * N) // TB  # 16
GMAX = 8             # kv chunks per exp group (PSUM: [128, GMAX*128] f32 = 2 banks)

_cache = {}


def _neighbors(visual_cortex_mask):
    flat = np.asarray(visual_cortex_mask).reshape(-1).astype(bool)
    act = np.flatnonzero(flat)
    if len(act) >= N:
        act = act[:N]
    else:
        act = np.concatenate([act, np.zeros(N - len(act), dtype=act.dtype)])
    vol = visual_cortex_mask.shape
    z, y, x = np.unravel_index(act, vol)
    coords = np.stack([z, y, x], axis=1).astype(np.int64)
    # exact integer squared distances; stable argsort == jax.lax.top_k order
    d2 = ((coords[:, None, :] - coords[None, :, :]) ** 2).sum(-1)
    order = np.argsort(d2, axis=1, kind="stable")
    nbr = order[:, 1 : KNN + 1].astype(np.int64)
    return coords, nbr


def _bisect_perm(coords):
    def rec(idx):
        if len(idx) <= 128:
            return [idx]
        c = coords[idx]
        ax = int(np.argmax(c.max(0) - c.min(0)))
        o = idx[np.argsort(c[:, ax], kind="stable")]
        h = len(o) // 2
        return rec(o[:h]) + rec(o[h:])

    return np.concatenate(rec(np.arange(N)))


def _plan(coords, nbr):
    perm = _bisect_perm(coords)
    invperm = np.empty(N, dtype=np.int64)
    invperm[perm] = np.arange(N)
    nbrP = invperm[nbr[perm]]  # [N, KNN] neighbor positions in permuted space
    blocks = []
    for t in range(NT):
        blocks.append(np.unique(nbrP[t * 128 : (t + 1) * 128] // 128).astype(np.int64))
    offs = np.concatenate([[0], np.cumsum([len(b) for b in blocks])]).astype(np.int64)
    totch = int(offs[-1])
    # mask[chunk, w, j] = 1 iff token blocks[t][c]*128+w is a neighbor of query t*128+j
    mask = np.zeros((totch, 128, 128), dtype=np.float16)
    for t in range(NT):
        blk = blocks[t]
        sub = nbrP[t * 128 : (t + 1) * 128]              # [128, KNN]
        c = np.searchsorted(blk, sub // 128)             # chunk slot per neighbor
        w = sub % 128
        j = np.broadcast_to(np.arange(128)[:, None], sub.shape)
        mask[offs[t] + c, w, j] = 1.0
    return perm, blocks, offs, mask


def _build_program(blocks, offs, totch):
    import concourse.bass as bass
    import concourse.bacc as bacc
    import concourse.tile as tile
    import concourse.mybir as mybir
    from concourse import masks as cmasks
    from contextlib import ExitStack

    F32 = mybir.dt.float32
    F32R = mybir.dt.float32r
    F16 = mybir.dt.float16
    EXP = mybir.ActivationFunctionType.Exp

    nc = bacc.Bacc("TRN2", target_bir_lowering=False, debug=False)
    xt_d = nc.dram_tensor("xt", [D, B * N], F32, kind="ExternalInput")
    wt_d = nc.dram_tensor("wt", [D, 3 * 128], F32, kind="ExternalInput")
    mk_d = nc.dram_tensor("maskt", [totch, 128, 128], F16, kind="ExternalInput")
    out_d = nc.dram_tensor("out", [B * N, 128], F32, kind="ExternalOutput")
    met_d = nc.dram_tensor("metric", [128, N], F32, kind="ExternalOutput")

    with tile.TileContext(nc) as tc:
        with ExitStack() as ctx:
            cpool = ctx.enter_context(tc.tile_pool(name="const", bufs=1))
            wpool = ctx.enter_context(tc.tile_pool(name="w", bufs=1))
            xpool = ctx.enter_context(tc.tile_pool(name="xblk", bufs=2))
            qkv = ctx.enter_context(tc.tile_pool(name="qkv", bufs=1))
            vtp = ctx.enter_context(tc.tile_pool(name="vtmp", bufs=2))
            mpool = ctx.enter_context(tc.tile_pool(name="mask", bufs=3))
            ppool = ctx.enter_context(tc.tile_pool(name="prob", bufs=6))
            opool = ctx.enter_context(tc.tile_pool(name="outp", bufs=3))
            spool = ctx.enter_context(tc.tile_pool(name="small", bufs=4))
            ps_proj = ctx.enter_context(tc.tile_pool(name="psproj", bufs=2, space="PSUM"))
            ps_vt = ctx.enter_context(tc.tile_pool(name="psvt", bufs=1, space="PSUM"))
            ps_sc = ctx.enter_context(tc.tile_pool(name="pssc", bufs=2, space="PSUM"))
            ps_pv = ctx.enter_context(tc.tile_pool(name="pspv", bufs=1, space="PSUM"))

            ident = cpool.tile([128, 128], F16)
            cmasks.make_identity(nc, ident[:])

            # persistent SBUF state
            qT = qkv.tile([128, B * N], F16, tag="qT")    # [2 heads * 64, tok]
            kT = qkv.tile([128, B * N], F16, tag="kT")
            v_s = qkv.tile([128, B * NT, 2, 65], F16, tag="v")  # [tok%128, tile, head, hd+1]
            met = qkv.tile([128, N], F32, tag="met")      # [b*64+hd, n]
            nc.vector.memset(v_s[:, :, :, 64], 1.0)

            wt_s = wpool.tile([128, DC, 3 * 128], F32R)
            nc.gpsimd.dma_start(wt_s[:], wt_d.ap().rearrange("(c p) f -> p c f", p=128))

            # ---- projections ----
            for blk in range(NBLK):
                xb = xpool.tile([128, DC, TB], F32R, tag="xb")
                nc.gpsimd.dma_start(
                    xb[:],
                    xt_d.ap().rearrange("(c p) t -> p c t", p=128)[:, :, blk * TB : (blk + 1) * TB],
                )
                cols = slice(blk * TB, (blk + 1) * TB)
                for f in range(3):
                    ps = ps_proj.tile([128, TB], F32, tag="pp")
                    for dc in range(DC):
                        nc.tensor.matmul(
                            ps[:],
                            wt_s[:, dc, f * 128 : (f + 1) * 128],
                            xb[:, dc, :],
                            start=(dc == 0),
                            stop=(dc == DC - 1),
                        )
                    if f == 0:
                        nc.vector.tensor_copy(qT[:, cols], ps[:])
                    elif f == 1:
                        nc.vector.tensor_copy(kT[:, cols], ps[:])
                        b = (blk * TB) // N
                        mcols = slice(blk * TB - b * N, (blk + 1) * TB - b * N)
                        mtmp = vtp.tile([64, TB], F32, tag="mtmp")
                        nc.vector.tensor_copy(mtmp[:], ps[0:64, :])
                        nc.vector.tensor_add(
                            met[b * 64 : (b + 1) * 64, mcols], mtmp[:], ps[64:128, :]
                        )
                    else:
                        vt = vtp.tile([128, TB], F16, tag="vt")
                        nc.vector.tensor_copy(vt[:], ps[:])
                        for j in range(TB // 128):
                            tp = ps_vt.tile([128, 128], F16, tag="tp")
                            nc.tensor.transpose(tp[:], vt[:, j * 128 : (j + 1) * 128], ident[:])
                            tidx = blk * (TB // 128) + j
                            nc.vector.tensor_copy(
                                v_s[:, tidx, :, 0:64],
                                tp[:].rearrange("p (h e) -> p h e", h=2),
                            )

            # ---- block-sparse attention ----
            for t in range(NT):
                blk_list = [int(x) for x in blocks[t]]
                s_t = len(blk_list)
                off = int(offs[t])
                mk = mpool.tile([128, GMAX * 2, 128], F16, tag="mk")
                nc.sync.dma_start(
                    mk[:, :s_t, :],
                    mk_d.ap()[off : off + s_t].rearrange("c p j -> p c j"),
                )
                groups = [
                    blk_list[g : g + GMAX] for g in range(0, s_t, GMAX)
                ]
                for b in range(B):
                    qcols = slice(b * N + t * 128, b * N + t * 128 + 128)
                    obt = opool.tile([128, 128], F32, tag="obt")
                    for h in range(2):
                        hrows = slice(h * 64, (h + 1) * 64)
                        ptiles = []
                        for gi, grp in enumerate(groups):
                            sg = len(grp)
                            sp = ps_sc.tile([128, GMAX, 128], F32, tag="sc")
                            for ci, bk in enumerate(grp):
                                kcols = slice(b * N + bk * 128, b * N + bk * 128 + 128)
                                nc.tensor.matmul(
                                    sp[:, ci, :], kT[hrows, kcols], qT[hrows, qcols],
                                    start=True, stop=True,
                                )
                            pt = ppool.tile([128, GMAX, 128], F16, tag="pt")
                            nc.scalar.activation(pt[:, :sg, :], sp[:, :sg, :], EXP, scale=0.125)
                            nc.vector.tensor_mul(
                                pt[:, :sg, :], pt[:, :sg, :],
                                mk[:, gi * GMAX : gi * GMAX + sg, :],
                            )
                            ptiles.append(pt)
                        pv = ps_pv.tile([128, 65], F32, tag="pv")
                        n_mm = 0
                        for gi, grp in enumerate(groups):
                            for ci, bk in enumerate(grp):
                                nc.tensor.matmul(
                                    pv[:],
                                    ptiles[gi][:, ci, :],
                                    v_s[:, b * NT + bk, h, :],
                                    start=(n_mm == 0),
                                    stop=(n_mm == s_t - 1),
                                )
                                n_mm += 1
                        rr = spool.tile([128, 1], F32, tag="rr")
                        nc.vector.reciprocal(rr[:], pv[:, 64:65])
                        nc.vector.tensor_scalar_mul(
                            obt[:, h * 64 : (h + 1) * 64], pv[:, 0:64], rr[:]
                        )
                    nc.sync.dma_start(
                        out_d.ap().rearrange("(T p) f -> T p f", p=128)[b * NT + t],
                        obt[:],
                    )

            nc.sync.dma_start(met_d.ap(), met[:])

    nc.compile()
    return nc


def kernel(x, visual_cortex_mask, Wq, Wk, Wv):
    from concourse import bass_utils

    x = np.asarray(x, dtype=np.float32)
    Wq = np.asarray(Wq, dtype=np.float32)
    Wk = np.asarray(Wk, dtype=np.float32)
    Wv = np.asarray(Wv, dtype=np.float32)

    coords, nbr = _neighbors(visual_cortex_mask)
    key = (coords.tobytes(), nbr.tobytes())
    if key not in _cache:
        perm, blocks, offs, mask = _plan(coords, nbr)
        nc = _build_program(blocks, offs, mask.shape[0])
        _cache[key] = (perm, mask, nc)
    perm, mask, nc = _cache[key]

    xP = x[:, perm, :]                                   # [B, N, D]
    xT = np.ascontiguousarray(xP.reshape(B * N, D).T)    # [D, B*N]

    in_maps = []
    for c in range(NCORES):
        rows = slice(c * 128, (c + 1) * 128)
        wt = np.ascontiguousarray(
            np.concatenate([Wq[rows], Wk[rows], Wv[rows]], axis=0).T
        )  # [D, 384]
        in_maps.append({"xt": xT, "wt": wt, "maskt": mask})

    res = bass_utils.run_bass_kernel_spmd(nc, in_maps, core_ids=list(range(NCORES)))

    out = np.empty((B, N, H, HD), dtype=np.float32)
    metric_acc = np.zeros((128, N), dtype=np.float32)
    for c in range(NCORES):
        oc = res.results[c]["out"].reshape(B, N, 2, HD)
        out[:, :, 2 * c, :] = oc[:, :, 0, :]
        out[:, :, 2 * c + 1, :] = oc[:, :, 1, :]
        metric_acc += res.results[c]["metric"]
    metric_acc /= H

    out_full = np.empty((B, N, D), dtype=np.float32)
    out_full[:, perm, :] = out.reshape(B, N, D)
    metric_full = np.empty((B, N, HD), dtype=np.float32)
    for b in range(B):
        metric_full[b, perm, :] = metric_acc[b * 64 : (b + 1) * 64, :].T
    return out_full, metric_full


# revision 7
# speedup vs baseline: 1.0406x; 1.0406x over previous
"""Nearest-neighbor attention Trainium2 kernel (8 NeuronCores).

Strategy:
  - Host: compute kNN neighbor lists from the voxel mask (tiny), pick a
    spatial ordering (recursive bisection) of the 4096 queries, and build
    per-128-query-tile lists of 128-token kv blocks containing neighbors,
    plus {0,1} masks selecting true neighbor entries inside those blocks.
  - Device (SPMD over 8 cores, tensor-parallel over heads: 2 heads/core):
    QKV projections as float32r matmuls (x pre-transposed on host), then
    block-sparse attention: scores^T = kT.T-chunk @ qT-tile per kv block,
    exp via ScalarE, mask multiply, PV via matmul with an appended
    ones-column for the softmax denominator, then per-row normalization.
  - Host: gather per-core outputs, undo the query permutation.
"""

import sys
import os
import numpy as np

sys.path.insert(0, "/opt/trn_rl_repo")

B, N, D = 2, 4096, 1024
H, KNN = 16, 16
HD = D // H          # 64
NCORES = 8
HPC = H // NCORES    # heads per core = 2
NT = N // 128        # 32 query tiles per batch
DC = D // 128        # 8 contraction chunks
TB = 512             # projection token-block size
NBLK = (B * N) // TB  # 16
GMAX = 8             # kv chunks per exp group (PSUM: [128, GMAX*128] f32 = 2 banks)

_cache = {}


def _neighbors(visual_cortex_mask):
    flat = np.asarray(visual_cortex_mask).reshape(-1).astype(bool)
    act = np.flatnonzero(flat)
    if len(act) >= N:
        act = act[:N]
    else:
        act = np.concatenate([act, np.zeros(N - len(act), dtype=act.dtype)])
    vol = visual_cortex_mask.shape
    z, y, x = np.unravel_index(act, vol)
    coords = np.stack([z, y, x], axis=1).astype(np.int64)
    # exact integer squared distances; stable argsort == jax.lax.top_k order
    d2 = ((coords[:, None, :] - coords[None, :, :]) ** 2).sum(-1)
    order = np.argsort(d2, axis=1, kind="stable")
    nbr = order[:, 1 : KNN + 1].astype(np.int64)
    return coords, nbr


def _bisect_perm(coords):
    def rec(idx):
        if len(idx) <= 128:
            return [idx]
        c = coords[idx]
        ax = int(np.argmax(c.max(0) - c.min(0)))
        o = idx[np.argsort(c[:, ax], kind="stable")]
        h = len(o) // 2
        return rec(o[:h]) + rec(o[h:])

    return np.concatenate(rec(np.arange(N)))


def _plan(coords, nbr):
    perm = _bisect_perm(coords)
    invperm = np.empty(N, dtype=np.int64)
    invperm[perm] = np.arange(N)
    nbrP = invperm[nbr[perm]]  # [N, KNN] neighbor positions in permuted space
    blocks = []
    for t in range(NT):
        blocks.append(np.unique(nbrP[t * 128 : (t + 1) * 128] // 128).astype(np.int64))
    offs = np.concatenate([[0], np.cumsum([len(b) for b in blocks])]).astype(np.int64)
    totch = int(offs[-1])
    # mask[chunk, w, j] = 1 iff token blocks[t][c]*128+w is a neighbor of query t*128+j
    mask = np.zeros((totch, 128, 128), dtype=np.float16)
    for t in range(NT):
        blk = blocks[t]
        sub = nbrP[t * 128 : (t + 1) * 128]              # [128, KNN]
        c = np.searchsorted(blk, sub // 128)             # chunk slot per neighbor
        w = sub % 128
        j = np.broadcast_to(np.arange(128)[:, None], sub.shape)
        mask[offs[t] + c, w, j] = 1.0
    return perm, blocks, offs, mask


def _build_program(blocks, offs, totch):
    import concourse.bass as bass
    import concourse.bacc as bacc
    import concourse.tile as tile
    import concourse.mybir as mybir
    from concourse import masks as cmasks
    from contextlib import ExitStack

    F32 = mybir.dt.float32
    F32R = mybir.dt.float32r
    F16 = mybir.dt.float16
    EXP = mybir.ActivationFunctionType.Exp

    nc = bacc.Bacc("TRN2", target_bir_lowering=False, debug=False)
    xt_d = nc.dram_tensor("xt", [D, B * N], F32, kind="ExternalInput")
    wt_d = nc.dram_tensor("wt", [D, 3 * 128], F32, kind="ExternalInput")
    mk_d = nc.dram_tensor("maskt", [totch, 128, 128], F16, kind="ExternalInput")
    out_d = nc.dram_tensor("out", [B * N, 128], F32, kind="ExternalOutput")
    met_d = nc.dram_tensor("metric", [128, N], F32, kind="ExternalOutput")

    SB = 4  # projection blocks per supergroup (stationary reuse)

    with tile.TileContext(nc) as tc:
        with ExitStack() as ctx:
            cpool = ctx.enter_context(tc.tile_pool(name="const", bufs=1))
            wpool = ctx.enter_context(tc.tile_pool(name="w", bufs=1))
            qkv = ctx.enter_context(tc.tile_pool(name="qkv", bufs=1))
            mpool = ctx.enter_context(tc.tile_pool(name="mask", bufs=3))
            ppool = ctx.enter_context(tc.tile_pool(name="prob", bufs=6))
            opool = ctx.enter_context(tc.tile_pool(name="outp", bufs=3))
            spool = ctx.enter_context(tc.tile_pool(name="small", bufs=4))

            ident = cpool.tile([128, 128], F16)
            cmasks.make_identity(nc, ident[:])

            # persistent SBUF state
            qT = qkv.tile([128, B * N], F16, tag="qT")    # [2 heads * 64, tok]
            kT = qkv.tile([128, B * N], F16, tag="kT")
            v_s = qkv.tile([128, B * NT, 2, 65], F16, tag="v")  # [tok%128, tile, head, hd+1]
            met = qkv.tile([128, N], F32, tag="met")      # [b*64+hd, n]
            nc.vector.memset(v_s[:, :, :, 64], 1.0)

            wt_s = wpool.tile([128, DC, 3 * 128], F32R)
            nc.gpsimd.dma_start(wt_s[:], wt_d.ap().rearrange("(c p) f -> p c f", p=128))

            # ---- projections (stationary-reuse supergroups) ----
            with ExitStack() as pctx:
                xpool = pctx.enter_context(tc.tile_pool(name="xblk", bufs=SB + 1))
                vtp = pctx.enter_context(tc.tile_pool(name="vtmp", bufs=2))
                ps_proj = [
                    pctx.enter_context(tc.tile_pool(name=f"pspj{i}", bufs=1, space="PSUM"))
                    for i in range(SB)
                ]
                ps_vt = pctx.enter_context(tc.tile_pool(name="psvt", bufs=2, space="PSUM"))

                for sb in range(0, NBLK, SB):
                    xbs = []
                    for blk in range(sb, sb + SB):
                        xb = xpool.tile([128, DC, TB], F32R, tag="xb")
                        nc.gpsimd.dma_start(
                            xb[:],
                            xt_d.ap().rearrange("(c p) t -> p c t", p=128)[
                                :, :, blk * TB : (blk + 1) * TB
                            ],
                        )
                        xbs.append(xb)
                    for f in range(3):
                        pss = [
                            ps_proj[i].tile([128, TB], F32, tag=f"pp{i}", name=f"pp{i}")
                            for i in range(SB)
                        ]
                        for dc in range(DC):
                            for i in range(SB):
                                nc.tensor.matmul(
                                    pss[i][:],
                                    wt_s[:, dc, f * 128 : (f + 1) * 128],
                                    xbs[i][:, dc, :],
                                    start=(dc == 0),
                                    stop=(dc == DC - 1),
                                )
                        for i in range(SB):
                            blk = sb + i
                            ps = pss[i]
                            cols = slice(blk * TB, (blk + 1) * TB)
                            if f == 0:
                                nc.vector.tensor_copy(qT[:, cols], ps[:])
                            elif f == 1:
                                nc.vector.tensor_copy(kT[:, cols], ps[:])
                                b = (blk * TB) // N
                                mcols = slice(blk * TB - b * N, (blk + 1) * TB - b * N)
                                mtmp = vtp.tile([64, TB], F32, tag="mtmp")
                                nc.vector.tensor_copy(mtmp[:], ps[0:64, :])
                                nc.vector.tensor_add(
                                    met[b * 64 : (b + 1) * 64, mcols], mtmp[:], ps[64:128, :]
                                )
                            else:
                                vt = vtp.tile([128, TB], F16, tag="vt")
                                nc.vector.tensor_copy(vt[:], ps[:])
                                for j in range(TB // 128):
                                    tp = ps_vt.tile([128, 128], F16, tag="tp")
                                    nc.tensor.transpose(
                                        tp[:], vt[:, j * 128 : (j + 1) * 128], ident[:]
                                    )
                                    tidx = blk * (TB // 128) + j
                                    nc.vector.tensor_copy(
                                        v_s[:, tidx, :, 0:64],
                                        tp[:].rearrange("p (h e) -> p h e", h=2),
                                    )

            # ---- block-sparse attention (scores head-packed via row groups) ----
            with ExitStack() as actx:
                ps_sc = actx.enter_context(tc.tile_pool(name="pssc", bufs=3, space="PSUM"))
                ps_pv = actx.enter_context(tc.tile_pool(name="pspv", bufs=2, space="PSUM"))

                for t in range(NT):
                    blk_list = [int(x) for x in blocks[t]]
                    s_t = len(blk_list)
                    off = int(offs[t])
                    mk = mpool.tile([128, GMAX * 2, 128], F16, tag="mk")
                    nc.sync.dma_start(
                        mk[:, :s_t, :],
                        mk_d.ap()[off : off + s_t].rearrange("c p j -> p c j"),
                    )
                    groups = [blk_list[g : g + GMAX] for g in range(0, s_t, GMAX)]
                    for b in range(B):
                        qcols = slice(b * N + t * 128, b * N + t * 128 + 128)
                        obt = opool.tile([128, 128], F32, tag="obt")
                        ptiles = {0: [], 1: []}
                        for gi, grp in enumerate(groups):
                            sg = len(grp)
                            sps = [
                                ps_sc.tile([128, GMAX, 128], F32, tag="sc", name="sc")
                                for _ in range(2)
                            ]
                            for ci, bk in enumerate(grp):
                                kcols = slice(b * N + bk * 128, b * N + bk * 128 + 128)
                                for h in range(2):
                                    hrows = slice(h * 64, (h + 1) * 64)
                                    nc.tensor.matmul(
                                        sps[h][:, ci, :], kT[hrows, kcols],
                                        qT[hrows, qcols], start=True, stop=True,
                                    )
                            for h in range(2):
                                pt = ppool.tile([128, GMAX, 128], F16, tag="pt")
                                nc.scalar.activation(
                                    pt[:, :sg, :], sps[h][:, :sg, :], EXP, scale=0.125
                                )
                                nc.vector.tensor_mul(
                                    pt[:, :sg, :], pt[:, :sg, :],
                                    mk[:, gi * GMAX : gi * GMAX + sg, :],
                                )
                                ptiles[h].append(pt)
                        for h in range(2):
                            pv = ps_pv.tile([128, 65], F32, tag="pv")
                            n_mm = 0
                            for gi, grp in enumerate(groups):
                                for ci, bk in enumerate(grp):
                                    nc.tensor.matmul(
                                        pv[:],
                                        ptiles[h][gi][:, ci, :],
                                        v_s[:, b * NT + bk, h, :],
                                        start=(n_mm == 0),
                                        stop=(n_mm == s_t - 1),
                                    )
                                    n_mm += 1
                            rr = spool.tile([128, 1], F32, tag="rr")
                            nc.vector.reciprocal(rr[:], pv[:, 64:65])
                            nc.vector.tensor_scalar_mul(
                                obt[:, h * 64 : (h + 1) * 64], pv[:, 0:64], rr[:]
                            )
                        nc.sync.dma_start(
                            out_d.ap().rearrange("(T p) f -> T p f", p=128)[b * NT + t],
                            obt[:],
                        )

            nc.sync.dma_start(met_d.ap(), met[:])

    nc.compile()
    return nc


def kernel(x, visual_cortex_mask, Wq, Wk, Wv):
    from concourse import bass_utils

    x = np.asarray(x, dtype=np.float32)
    Wq = np.asarray(Wq, dtype=np.float32)
    Wk = np.asarray(Wk, dtype=np.float32)
    Wv = np.asarray(Wv, dtype=np.float32)

    coords, nbr = _neighbors(visual_cortex_mask)
    key = (coords.tobytes(), nbr.tobytes())
    if key not in _cache:
        perm, blocks, offs, mask = _plan(coords, nbr)
        nc = _build_program(blocks, offs, mask.shape[0])
        _cache[key] = (perm, mask, nc)
    perm, mask, nc = _cache[key]

    xP = x[:, perm, :]                                   # [B, N, D]
    xT = np.ascontiguousarray(xP.reshape(B * N, D).T)    # [D, B*N]

    in_maps = []
    for c in range(NCORES):
        rows = slice(c * 128, (c + 1) * 128)
        wt = np.ascontiguousarray(
            np.concatenate([Wq[rows], Wk[rows], Wv[rows]], axis=0).T
        )  # [D, 384]
        in_maps.append({"xt": xT, "wt": wt, "maskt": mask})

    res = bass_utils.run_bass_kernel_spmd(nc, in_maps, core_ids=list(range(NCORES)))

    out = np.empty((B, N, H, HD), dtype=np.float32)
    metric_acc = np.zeros((128, N), dtype=np.float32)
    for c in range(NCORES):
        oc = res.results[c]["out"].reshape(B, N, 2, HD)
        out[:, :, 2 * c, :] = oc[:, :, 0, :]
        out[:, :, 2 * c + 1, :] = oc[:, :, 1, :]
        metric_acc += res.results[c]["metric"]
    metric_acc /= H

    out_full = np.empty((B, N, D), dtype=np.float32)
    out_full[:, perm, :] = out.reshape(B, N, D)
    metric_full = np.empty((B, N, HD), dtype=np.float32)
    for b in range(B):
        metric_full[b, perm, :] = metric_acc[b * 64 : (b + 1) * 64, :].T
    return out_full, metric_full
